# revision 69
# baseline (speedup 1.0000x reference)
"""CPI_DGLLife kernel for 8 Trainium2 NeuronCores (SPMD).

GCN over a 65536-node graph + protein conv1d branch + CPI head.
Sharding: data-parallel over the 512-graph batch (64 graphs / core).

v2 design (vs baseline):
- Single gather stream: bf16 node table packed as 32768 x 512B granules
  (2 node rows each) so int16 indices cover all 65536 nodes. Exact
  per-edge tokens (no 3-table split padding); pad slots carry weight 0.
- Readout commuted past the per-graph segment sum: W_ro_in/W_ro_out are
  applied to the [64, 128] graph sums instead of per node (linear ops
  commute with segment_sum; node-count bias term handled separately).
- bf16 everywhere on the matmul path (fp32 PSUM accumulate), which
  halves gather HBM traffic and runs matmuls at 1 cycle/row.
- Conv restructured: layer-1 folds embed@K1 into a 75-row tap-stacked
  one-hot matmul (1 pass instead of 3); acts split Scalar/Vector.
"""
import sys
sys.path.insert(0, "/opt/trn_rl_repo")
import contextlib
import numpy as np
import ml_dtypes

import concourse.bass as bass
import concourse.bacc as bacc
import concourse.tile as tile
from concourse import mybir
from concourse.ap import AP as APraw
from concourse.bass_utils import run_bass_kernel_spmd
from concourse.masks import make_identity

dt = mybir.dt
AF = mybir.ActivationFunctionType
ALU = mybir.AluOpType
AX = mybir.AxisListType
DR = mybir.MatmulPerfMode.DoubleRow
BF16 = np.dtype(ml_dtypes.bfloat16)
F8E4 = np.dtype(ml_dtypes.float8_e4m3)

P = 128
N, E, B, L = 65536, 262144, 512, 1000
IN_DIM, HID, VOCAB = 74, 128, 25
CHANNELS = [HID, 96, 128, IN_DIM, HID]
NCORES = 8
GPC = B // NCORES              # graphs per core = 64
PPC = GPC                      # proteins per core = 64
CB = 24                        # gather chunk budget (128-token blocks)
LCONV = 1002                   # 1000 + 2 guard cols


# ------------------------------------------------------------------ host prep
def _host_prep(inputs):
    graph_ids = np.asarray(inputs["graph_ids"])
    src = np.concatenate([np.asarray(inputs["edge_src"]).astype(np.int64),
                          np.arange(N, dtype=np.int64)])
    dst = np.concatenate([np.asarray(inputs["edge_dst"]).astype(np.int64),
                          np.arange(N, dtype=np.int64)])
    deg_out = np.bincount(src, minlength=N).astype(np.float32)
    deg_in = np.bincount(dst, minlength=N).astype(np.float32)

    core_node_lo = np.searchsorted(graph_ids, np.arange(0, B + 1, GPC))
    ncore_nodes = core_node_lo[1:] - core_node_lo[:-1]
    NT = int(np.ceil(ncore_nodes.max() / P))
    NPAD = NT * P

    # per-core degree-sorted node permutation (padded with -1)
    perm = np.full((NCORES, NPAD), -1, np.int64)
    for c in range(NCORES):
        lo, hi = int(core_node_lo[c]), int(core_node_lo[c + 1])
        order = np.argsort(-deg_in[lo:hi], kind="stable") + lo
        perm[c, :hi - lo] = order

    # S tiles: [NT, P, GPC] graph membership of permuted nodes
    S = np.zeros((NCORES, NT, P, GPC), np.float32)
    for c in range(NCORES):
        pm = perm[c]
        valid = pm >= 0
        g = graph_ids[pm[valid]] - c * GPC
        tt = np.arange(NPAD)[valid] // P
        pp = np.arange(NPAD)[valid] % P
        S[c, tt, pp, g] = 1.0

    # node -> (core, tile-position) in permuted order
    pos_of = np.full(N, -1, np.int64)
    core_of = np.full(N, -1, np.int64)
    for c in range(NCORES):
        pm = perm[c]
        v = pm >= 0
        pos_of[pm[v]] = np.arange(NPAD)[v]
        core_of[pm[v]] = c

    # edge -> (core, tile, lane) from dst; slot = rank within (c,t,p)
    ec = core_of[dst]
    et = pos_of[dst] // P
    ep = pos_of[dst] % P
    key = (ec * NT + et) * P + ep
    order = np.argsort(key, kind="stable")
    ks = key[order]
    starts = np.r_[0, np.flatnonzero(np.diff(ks)) + 1]
    grp_len = np.diff(np.r_[starts, len(ks)])
    slot_sorted = np.arange(len(ks)) - np.repeat(starts, grp_len)
    slot = np.empty(len(ks), np.int64)
    slot[order] = slot_sorted
    cnt = np.zeros(NCORES * NT * P, np.int64)
    uk, uc = np.unique(ks, return_counts=True)
    cnt[uk] = uc
    kmax = cnt.reshape(NCORES, NT, P).max(axis=(0, 2))  # [NT] shared schedule
    Bpref = np.r_[0, np.cumsum(kmax)]
    NBLK = int(Bpref[-1])
    NTOK = NBLK * P

    # tile order in the token stream: ascending kmax (small tiles first so
    # the first gather chunks are small and land early)
    tile_order = [t for t in range(NT)][::-1]
    Bof = np.zeros(NT, np.int64)
    off = 0
    for t in tile_order:
        Bof[t] = off
        off += kmax[t]
    assert off == NBLK

    # token (t, slot, lane) -> flat position; idx = src granule, pad -> 0
    tokpos = (Bof[et] + slot) * P + ep
    idx_flat = np.zeros((NCORES, NTOK), np.int16)
    idx_flat[ec, tokpos] = (src >> 1).astype(np.int16)
    # per-token 2-half weights: deg product at the src half, 1e30 (w~0) else
    dval = np.full((NCORES, P, NBLK, 2), 1e30, np.float32)
    dval[ec, ep, Bof[et] + slot, src & 1] = deg_out[src] * deg_in[dst]

    def wrap(a):  # token-major [NCORES, NTOK] -> wrapped [NCORES, 128, NTOK//16]
        ncol = a.shape[1] // 16
        w = a.reshape(NCORES, ncol, 16).transpose(0, 2, 1)
        return np.ascontiguousarray(np.tile(w, (1, 8, 1)))

    idx_wrapped = wrap(idx_flat)

    # gather chunks: pack whole tiles per instruction, ramped budgets so the
    # first chunks finish their descriptor-gen + transfer quickly
    budgets = [8, 8, 8, 8, 16, 16, 16, 16]
    chunks = []
    cur, cb, b0 = [], 0, 0
    for t in tile_order:
        k = int(kmax[t])
        if k == 0:
            continue
        budget = budgets[len(chunks)] if len(chunks) < len(budgets) else CB
        if cb + k > budget and cur:
            chunks.append((cur, b0, cb))
            b0 += cb
            cur, cb = [], 0
        cur.append((t, k))
        cb += k
    if cur:
        chunks.append((cur, b0, cb))

    # bf16 node table: 2 rows (2x 128 cols) per 512B granule
    tab = np.zeros((N, P), BF16)
    tab[:, :IN_DIM] = np.asarray(inputs["node_feats"], np.float32)
    tabg = np.ascontiguousarray(tab.reshape(N // 2, 2 * P))

    # tap-stacked protein one-hot: oh3[c, p, 25t+v, j] = [seq[j+t-1] == v]
    seq = np.asarray(inputs["protein_seq"]).reshape(NCORES, PPC, L)
    oh3 = np.zeros((NCORES, PPC, 3 * VOCAB, L), np.float32)
    ci = np.arange(NCORES)[:, None, None]
    pi = np.arange(PPC)[None, :, None]
    for t in range(3):
        j = np.arange(max(0, 1 - t), min(L, L + 1 - t))
        vals = seq[:, :, j + t - 1]
        oh3[ci, pi, VOCAB * t + vals, j[None, None, :]] = 1.0
    oh3 = oh3.reshape(NCORES, PPC // 4, 4, 3 * VOCAB, L)
    oh3 = np.ascontiguousarray(oh3.transpose(0, 1, 3, 2, 4)).reshape(
        NCORES, PPC // 4, 3 * VOCAB, 4 * L).astype(BF16)

    n_g = np.bincount(graph_ids, minlength=B).astype(np.float32)
    n_g = n_g.reshape(NCORES, 1, GPC)

    # pre-arranged for contiguous DMA; graphs padded to 128 cols (FWL)
    S_r = np.zeros((NCORES, P, NT, P), np.float32)
    S_r[:, :, :, :GPC] = S.transpose(0, 2, 1, 3)

    f32 = np.float32
    shared = {
        "tabg": tabg,
        "W_gc": np.asarray(inputs["W_gc"], f32).astype(BF16),      # [74,128] rhs
        "W_ri": np.asarray(inputs["W_ro_in"], f32).astype(BF16),   # [128,128] lhsT
        "W_ro": np.asarray(inputs["W_ro_out"], f32).astype(BF16),
        "Wc1": np.asarray(inputs["Wc1"], f32).astype(BF16),
        "Wc2": np.asarray(inputs["Wc2"], f32).astype(BF16),
        "embedT": np.ascontiguousarray(
            np.asarray(inputs["embed"], f32).T).astype(BF16),      # [128, 25]
        "Wf1_r": np.ascontiguousarray(
            np.asarray(inputs["Wf1"], f32).reshape(2, HID, 2 * HID)
            .transpose(1, 0, 2)).astype(BF16),                     # [HID,2,2H]
        "bf1_r": np.ascontiguousarray(
            np.asarray(inputs["bf1"], f32).reshape(2, HID, 1)
            .transpose(1, 0, 2)),
        "Wf2_r": np.ascontiguousarray(
            np.asarray(inputs["Wf2"], f32).reshape(2, HID, 1)
            .transpose(1, 0, 2)).astype(BF16),
        "bf2": np.asarray(inputs["bf2"], f32).reshape(1, 1),
        "bc1": np.asarray(inputs["bc1"], f32).reshape(HID, 1),
        "bc2": np.asarray(inputs["bc2"], f32).reshape(HID, 1),
        "bgc_row": np.asarray(inputs["b_gc"], f32).reshape(1, HID).astype(BF16),
        "b1row": np.asarray(inputs["b_ro_in"], f32).reshape(1, HID).astype(BF16),
        "b2row": np.asarray(inputs["b_ro_out"], f32).reshape(1, HID).astype(BF16),
    }
    for l in range(4):
        K = np.asarray(inputs["K%d" % (l + 1)], f32)  # [o, i, 3]
        KT_ = np.ascontiguousarray(K.transpose(1, 2, 0))  # [i, 3, o]
        if KT_.shape[2] < P:  # pad stationary cols to 128 -> enables FWL
            KT_ = np.concatenate(
                [KT_, np.zeros((KT_.shape[0], 3, P - KT_.shape[2]), f32)],
                axis=2)
        shared["K%dT" % (l + 1)] = np.ascontiguousarray(KT_).astype(BF16)
        shared["cb%d" % (l + 1)] = np.asarray(
            inputs["cb%d" % (l + 1)], f32).reshape(-1, 1)


    percore = []
    for c in range(NCORES):
        percore.append({
            "S": np.ascontiguousarray(S_r[c]).astype(BF16),
            "oh3": np.ascontiguousarray(oh3[c]),
            "ixs": idx_wrapped[c],
            "dval": np.ascontiguousarray(dval[c]),
            "ngrow": np.ascontiguousarray(n_g[c]),
        })
    gc_bias = bool(np.any(np.asarray(inputs["b_gc"]) != 0))
    ro_bias = bool(np.any(np.asarray(inputs["b_ro_in"]) != 0)
                   or np.any(np.asarray(inputs["b_ro_out"]) != 0))
    meta = dict(NT=NT, NBLK=NBLK, NTOK=NTOK, chunks=chunks,
                gc_bias=gc_bias, ro_bias=ro_bias)
    return shared, percore, meta


# --------------------------------------------------------------- device build
def _build(shared, meta):
    NT = meta["NT"]
    NBLK = meta["NBLK"]
    NTOK = meta["NTOK"]
    chunks = meta["chunks"]
    maxblk = max(cb for (_, _, cb) in chunks)

    nc = bacc.Bacc("TRN2", target_bir_lowering=False, debug=False,
                   num_devices=NCORES, num_swdge_queues=4)
    f32, bf16, i16, f8 = dt.float32, dt.bfloat16, dt.int16, dt.float8e4

    D = {k: nc.dram_tensor(k, list(v.shape), dt.from_np(v.dtype),
                           kind="ExternalInput")
         for k, v in shared.items()}
    D["S"] = nc.dram_tensor("S", [P, NT, P], bf16, kind="ExternalInput")
    D["oh3"] = nc.dram_tensor("oh3", [PPC // 4, 3 * VOCAB, 4 * L], bf16,
                              kind="ExternalInput")
    D["ixs"] = nc.dram_tensor("ixs", [P, NTOK // 16], i16,
                              kind="ExternalInput")
    D["dval"] = nc.dram_tensor("dval", [P, NBLK, 2], f32,
                               kind="ExternalInput")
    D["ngrow"] = nc.dram_tensor("ngrow", [1, GPC], f32,
                                kind="ExternalInput")
    out_d = nc.dram_tensor("out", [1, GPC], f32, kind="ExternalOutput")

    with tile.TileContext(nc) as tc, contextlib.ExitStack() as ctx:
        wp = ctx.enter_context(tc.tile_pool(name="wp", bufs=1))
        cvp = ctx.enter_context(tc.tile_pool(name="cvp", bufs=1))
        gp = ctx.enter_context(tc.tile_pool(name="gp", bufs=1))
        gnp = ctx.enter_context(tc.tile_pool(name="gnp", bufs=2))
        pcv = ctx.enter_context(tc.tile_pool(name="pcv", bufs=5, space="PSUM"))
        pgn = ctx.enter_context(tc.tile_pool(name="pgn", bufs=1, space="PSUM"))
        hgp = ctx.enter_context(tc.tile_pool(name="hgp", bufs=1, space="PSUM"))

        # ---------------- setup: weights to SBUF
        def ld(name, shape, dtype, src=None):
            t = wp.tile(shape, dtype, tag=name)
            nc.sync.dma_start(out=t[:], in_=D[name][:] if src is None else src)
            return t

        # gather-critical loads first so DGE can start immediately
        ixs = ld("ixs", [P, NTOK // 16], i16)
        dvt = ld("dval", [P, NBLK, 2], f32)
        embT = ld("embedT", [HID, VOCAB], bf16)
        KT = [ld("K%dT" % (l + 1), [CHANNELS[l], 3, P], bf16)
              for l in range(4)]
        cb = [ld("cb%d" % (l + 1), [CHANNELS[l + 1], 1], f32)
              for l in range(4)]
        W_gc = ld("W_gc", [IN_DIM, HID], bf16)
        ident = wp.tile([P, P], f32, tag="ident")
        make_identity(nc, ident[:])
        if meta["gc_bias"]:
            bgc_row = ld("bgc_row", [1, HID], bf16)
            ones1 = wp.tile([1, P], bf16, tag="ones1")
            nc.vector.memset(ones1[:], 1.0)

        # conv x tiles (ping-pong), guard cols zeroed once
        xb = []
        for l in range(3):
            pair = []
            for j in range(2):
                t = cvp.tile([CHANNELS[l + 1], LCONV], bf16,
                             tag="xb%d_%d" % (l, j))
                nc.vector.memset(t[:, 0:1], 0.0)
                nc.vector.memset(t[:, LCONV - 1:LCONV], 0.0)
                pair.append(t)
            xb.append(pair)

        # token weights: w = rsqrt(deg_out*deg_in); pad 1e30 -> ~0
        wz = wp.tile([P, NBLK, 2], bf16, tag="wz")
        nc.vector.reciprocal(dvt[:], dvt[:])
        nc.scalar.activation(wz[:], dvt[:], AF.Sqrt)

        # L1 stacked weights: rows 25t..25t+24 = embed @ K1_t^T, 128 cols (FWL)
        L1w = wp.tile([3 * VOCAB, P], bf16, tag="l1w")
        nc.vector.memset(L1w[:, CHANNELS[1]:], 0.0)
        for t in range(3):
            pm = pgn.tile([VOCAB, P], f32, space="PSUM", tag="hp")
            nc.tensor.matmul(pm[:], embT[:], KT[0][:, t, :], start=True,
                             stop=True)
            m1t = gnp.tile([VOCAB, CHANNELS[1]], bf16, tag="m1t")
            nc.scalar.copy(m1t[:], pm[:, :CHANNELS[1]])
            nc.sync.dma_start(out=L1w[VOCAB * t:VOCAB * (t + 1), :CHANNELS[1]],
                              in_=m1t[:])

        chunkmax = wp.tile([P, 2, PPC], f32, tag="chunkmax")
        acc = {}
        tile_seq = []
        # GNN graph-sum accumulator, fed tile-by-tile after the conv loop
        hgps = hgp.tile([P, HID], f32, space="PSUM", tag="hg")
        gnn_count = [0]

        def gnn_tile(t):
            i = gnn_count[0]
            gnn_count[0] += 1
            tp = pgn.tile([IN_DIM, P], f32, space="PSUM", tag="tp")
            nc.tensor.transpose(tp[:], acc[t][:], ident[:])
            aT = gnp.tile([IN_DIM, P], bf16, tag="aT")
            nc.scalar.copy(aT[:], tp[:])
            hp = pgn.tile([P, HID], f32, space="PSUM", tag="hp")
            nc.tensor.matmul(hp[:], aT[:], W_gc[:], start=True,
                             stop=not meta["gc_bias"])
            if meta["gc_bias"]:
                nc.tensor.matmul(hp[:], ones1[:], bgc_row[:], start=False,
                                 stop=True)
            h = gnp.tile([P, HID], bf16, tag="h")
            nc.scalar.activation(h[:], hp[:], AF.Relu)
            nc.tensor.matmul(hgps[:], Sg[:, t, :], h[:], start=(i == 0),
                             stop=(i == NT - 1), skip_group_check=True)

        # ---------------- gather machinery
        gtiles = {}

        def emit_gather(j):
            tl, b0, nb = chunks[j]
            g = gp.tile([P, maxblk, 2 * P], bf16, tag="g%d" % (j % 4))
            nc.gpsimd.dma_gather(
                out_ap=g[:, :nb, :], in_ap=D["tabg"][:],
                idxs_ap=ixs[:, b0 * 8:(b0 + nb) * 8],
                num_idxs=nb * P, num_idxs_reg=nb * P, elem_size=2 * P,
                single_packet=False, queue_num=j % 4)
            gtiles[j] = g

        gscr = gp.tile([P, IN_DIM, 2 * CB], bf16, tag="gscr")

        def drain_chunk(j):
            tl, b0, nb = chunks[j]
            g = gtiles[j]
            # scheduler hint: keep drains behind the conv work of the same
            # epoch (the list scheduler's DGE model is ~20x optimistic, so
            # without this it front-loads drains and stalls Vector on the
            # gather DMA, which back-pressures the conv PSUM pool)
            with tc.tile_wait_until(ms=(16 + 14 * (j // 4)) / 1000.0):
                # weighted tokens written d-major so the reduce is unit-stride
                gv = g[:, :nb, :].rearrange("p k (a d) -> p (k a) d", a=2)
                wv = wz[:, b0:b0 + nb, :].rearrange("p k a -> p (k a)")
                nc.vector.tensor_tensor(
                    out=gscr[:, :, :2 * nb],
                    in0=gv[:, :, :IN_DIM].rearrange("p m d -> p d m"),
                    in1=wv[:, None, :].to_broadcast([P, IN_DIM, 2 * nb]),
                    op=ALU.mult)
                off = 0
                for (t, k) in tl:
                    a = wp.tile([P, IN_DIM], f32, tag="acc%d" % t)
                    acc[t] = a
                    nc.vector.tensor_reduce(
                        out=a[:],
                        in_=gscr[:, :, 2 * off:2 * (off + k)],
                        axis=AX.X, op=ALU.add)
                    off += k
                tile_seq.extend(t for t, _ in tl)
            if j + 4 < len(chunks):
                emit_gather(j + 4)

        for j in range(min(4, len(chunks))):
            emit_gather(j)
        drain_ptr = [0]

        def drain_due(p):
            while (drain_ptr[0] < len(chunks)
                   and p >= 6 + 3 * drain_ptr[0]):
                drain_chunk(drain_ptr[0])
                drain_ptr[0] += 1

        # ---------------- conv protein loop
        for p in range(PPC):
            grp, sub = p // 4, p % 4
            if sub == 0:
                # scalar-engine HWDGE ring: keeps the 600KB one-hot loads off
                # the Sync queue, which gates the startup-critical weight DMAs
                ohg = cvp.tile([3 * VOCAB, 4 * L], bf16, tag="oh%d" % (grp % 2))
                nc.scalar.dma_start(out=ohg[:], in_=D["oh3"][grp])
            base = sub * L
            x1, x2, x3 = xb[0][p % 2], xb[1][p % 2], xb[2][p % 2]
            for ch in range(2):
                c0 = ch * 500
                pp = pcv.tile([P, 500], f32, space="PSUM", tag="cps")
                nc.tensor.matmul(pp[:], L1w[:], ohg[:, base + c0:base + c0 + 500],
                                 start=True, stop=True)
                nc.scalar.activation(x1[:, 1 + c0:501 + c0],
                                     pp[:CHANNELS[1], :], AF.Relu,
                                     bias=cb[0][:])
            for ch in range(2):
                c0 = ch * 500
                pp = pcv.tile([P, 500], f32, space="PSUM", tag="cps")
                for t in range(3):
                    nc.tensor.matmul(pp[:], KT[1][:, t, :],
                                     x1[:, c0 + t:c0 + t + 500],
                                     start=(t == 0), stop=(t == 2))
                nc.scalar.activation(x2[:, 1 + c0:501 + c0], pp[:],
                                     AF.Relu, bias=cb[1][:])
            for ch in range(2):
                c0 = ch * 500
                pp = pcv.tile([P, 500], f32, space="PSUM", tag="cps")
                for t in range(3):
                    nc.tensor.matmul(pp[:], KT[2][:, t, :],
                                     x2[:, c0 + t:c0 + t + 500],
                                     start=(t == 0), stop=(t == 2))
                nc.scalar.activation(x3[:, 1 + c0:501 + c0],
                                     pp[:CHANNELS[3], :], AF.Relu,
                                     bias=cb[2][:])
            for ch in range(2):
                c0 = ch * 500
                pp = pcv.tile([P, 500], f32, space="PSUM", tag="cps")
                for t in range(3):
                    nc.tensor.matmul(pp[:], KT[3][:, t, :],
                                     x3[:, c0 + t:c0 + t + 500],
                                     start=(t == 0), stop=(t == 2))
                nc.vector.reduce_max(out=chunkmax[:, ch, p:p + 1],
                                     in_=pp[:, :500], axis=AX.X)
            drain_due(p)

        while drain_ptr[0] < len(chunks):
            drain_chunk(drain_ptr[0])
            drain_ptr[0] += 1

        # late loads: needed only by the readout/head phase
        Sg = ld("S", [P, NT, P], bf16)
        W_ri = ld("W_ri", [HID, HID], bf16)
        W_ro = ld("W_ro", [HID, HID], bf16)
        Wc1 = ld("Wc1", [HID, HID], bf16)
        Wc2 = ld("Wc2", [HID, HID], bf16)
        Wf1 = ld("Wf1_r", [HID, 2, 2 * HID], bf16)
        bf1 = ld("bf1_r", [HID, 2, 1], f32)
        Wf2 = ld("Wf2_r", [HID, 2, 1], bf16)
        bf2 = ld("bf2", [1, 1], f32)
        bc1 = ld("bc1", [HID, 1], f32)
        bc2 = ld("bc2", [HID, 1], f32)
        b1row = ld("b1row", [1, HID], bf16)
        b2row = ld("b2row", [1, HID], bf16)
        ngrow_f = ld("ngrow", [1, GPC], f32)
        ngrow = wp.tile([1, GPC], bf16, tag="ngrow_b")
        nc.scalar.copy(ngrow[:], ngrow_f[:])
        identb = wp.tile([GPC, GPC], bf16, tag="identb")
        nc.scalar.copy(identb[:], ident[:GPC, :GPC])

        # pmax = relu(max over positions + cb4)  [128, PPC] bf16
        mxt = wp.tile([P, PPC], f32, tag="mxt")
        nc.vector.tensor_reduce(out=mxt[:],
                                in_=chunkmax[:].rearrange("p c q -> p q c"),
                                axis=AX.X, op=ALU.max)
        pmax = wp.tile([P, PPC], bf16, tag="pmax")
        nc.scalar.activation(pmax[:], mxt[:], AF.Relu, bias=cb[3][:])

        for t in tile_seq:
            gnn_tile(t)
        assert gnn_count[0] == NT
        # readout: hg = relu((sum_h @ W_ri + n_g b1) @ W_ro + n_g b2)
        hg_s = gnp.tile([GPC, HID], bf16, tag="hg_s")
        nc.scalar.copy(hg_s[:], hgps[:GPC, :])
        sT_ps = pgn.tile([HID, GPC], bf16, space="PSUM", tag="tp")
        nc.tensor.transpose(sT_ps[:], hg_s[:], identb[:])
        sT = gnp.tile([HID, GPC], bf16, tag="sT")
        nc.scalar.copy(sT[:], sT_ps[:])
        u_ps = pgn.tile([HID, GPC], f32, space="PSUM", tag="hp")
        nc.tensor.matmul(u_ps[:], W_ri[:], sT[:], start=True,
                         stop=not meta["ro_bias"])
        if meta["ro_bias"]:
            nc.tensor.matmul(u_ps[:], b1row[:], ngrow[:], start=False,
                             stop=True)
        u = gnp.tile([HID, GPC], bf16, tag="u")
        nc.scalar.copy(u[:], u_ps[:])
        v_ps = pgn.tile([HID, GPC], f32, space="PSUM", tag="hp")
        nc.tensor.matmul(v_ps[:], W_ro[:], u[:], start=True,
                         stop=not meta["ro_bias"])
        if meta["ro_bias"]:
            nc.tensor.matmul(v_ps[:], b2row[:], ngrow[:], start=False,
                             stop=True)
        hg = gnp.tile([HID, GPC], bf16, tag="hgv")
        nc.scalar.activation(hg[:], v_ps[:], AF.Relu)
        # compound FC
        c1ps = pgn.tile([HID, GPC], f32, space="PSUM", tag="hp")
        nc.tensor.matmul(c1ps[:], Wc1[:], hg[:], start=True, stop=True)
        cv1 = gnp.tile([HID, GPC], bf16, tag="cv1")
        nc.scalar.activation(cv1[:], c1ps[:], AF.Relu, bias=bc1[:])
        c2ps = pgn.tile([HID, GPC], f32, space="PSUM", tag="hp")
        nc.tensor.matmul(c2ps[:], Wc2[:], cv1[:], start=True, stop=True)
        cv2 = gnp.tile([HID, GPC], bf16, tag="cv2")
        nc.scalar.activation(cv2[:], c2ps[:], AF.Relu, bias=bc2[:])
        # CPI head
        zin = [cv2, pmax]
        z2 = []
        for mc in range(2):
            zps = pgn.tile([HID, GPC], f32, space="PSUM", tag="hp")
            for kc in range(2):
                nc.tensor.matmul(zps[:], Wf1[:, kc, mc * HID:(mc + 1) * HID],
                                 zin[kc][:, :GPC], start=(kc == 0),
                                 stop=(kc == 1))
            zt = gnp.tile([HID, GPC], bf16, tag="z2_%d" % mc)
            nc.scalar.activation(zt[:], zps[:], AF.Relu, bias=bf1[:, mc, :])
            z2.append(zt)
        ops = pgn.tile([1, GPC], f32, space="PSUM", tag="hp")
        for kc in range(2):
            nc.tensor.matmul(ops[:], Wf2[:, kc, :], z2[kc][:],
                             start=(kc == 0), stop=(kc == 1))
        ot = wp.tile([1, GPC], f32, tag="ot")
        nc.scalar.activation(ot[:], ops[:], AF.Sigmoid, bias=bf2[:1, :])
        nc.sync.dma_start(out=out_d[:], in_=ot[:])

    nc.compile()
    return nc


def kernel(**inputs):
    shared, percore, meta = _host_prep(inputs)
    nc = _build(shared, meta)
    in_maps = []
    for c in range(NCORES):
        m = dict(shared)
        m.update(percore[c])
        in_maps.append(m)
    res = run_bass_kernel_spmd(nc, in_maps, list(range(NCORES)))
    out = np.concatenate([res.results[c]["out"].reshape(GPC)
                          for c in range(NCORES)])
    return out.reshape(B, 1).astype(np.float32)


if __name__ == "__main__":
    sys.path.insert(0, "/root/problem")
    import jax
    import reference
    with jax.default_device(jax.devices("cpu")[0]):
        inputs = {k: np.asarray(v) for k, v in reference.setup_inputs().items()}
        exp = np.asarray(reference.reference(**inputs))
    got = kernel(**inputs)
    err = np.abs(got - exp).max()
    rel = err / max(np.abs(exp).max(), 1e-9)
    print("max abs err:", err, " rel:", rel)
